# revision 18
# baseline (speedup 1.0000x reference)
"""PixelAttention Trainium2 kernel.

Computes, for each batch image (data-parallel, one image per NeuronCore):
    seq  = image.reshape(C, T).T            # [T, C], T = 32*32
    kqv  = seq @ w_kqv + b_kqv
    per-head causal attention (8 heads, head_dim 32), softmax over keys
    out  = mix(attn) + b_mix + image

The ScalarE exp stream (~4.7M causal logits at 1 elem/cycle/lane) is the
roofline; everything else is organized to hide under it:
  - ScalarE runs nothing but the 48 exp calls (+1 warm-up that triggers the
    ACT table load at t~0 from a memset input).
  - QK^T logits land transposed L[s, t]; 4 heads row-packed (tile_position)
    per s-tile; the lp PSUM pool (2 slots x 2 banks) is reserved for the
    QK->exp stream alone.
  - AV matmuls ride a 2-deep FIFO, emitted after the next-but-one pair's
    QK, so exp->mask->AV chains never sit between an exp and the following
    QK on the in-order PE.
  - All non-QK PSUM transients (V projection, woven kq projections,
    denominator gather, reciprocal broadcast, mix) use a separate 1-bank
    "dv" pool; the work is spread across tiles via an `extras` schedule
    that lands in PE-idle windows, balanced against each phase's exp time.
  - Causal diag-block masking = GpSimd multiply by a 0/1 triangle.
  - AV accumulates [V | 1] (ones row = softmax denominator) col-packed
    2 heads/matmul; normalization (cast, gather-matmul, reciprocal,
    selector broadcast, multiply) + padded mix are staggered so only the
    last group's tail is exposed.
  - Head loads: one packed DMA (x + all projection weights, 6KB/partition
    lines); dummy matmuls warm the PE clock gate during the transfer; late
    constants are fetched mid-stream behind a data dependency.
"""

import numpy as np
import ml_dtypes

import concourse.bass as bass
import concourse.tile as tile
from concourse import bacc, mybir
from concourse.bass_utils import run_bass_kernel_spmd

BF = ml_dtypes.bfloat16
T, C, H, D = 1024, 256, 8, 32
N_CORES = 8

# hx1 (bf16): x half0 (1024) | wk kh0 (128) | wq kh0 (128) | bjt 4 | bm2 2
HX1_F = 1286
# hx2 (bf16): x half1 (1024) | wk kh1 (128) | wq kh1 (128) | wk2 [2,128] | wq2 [2,128]
HX2_F = 1792
# cb_early packed layout (bf16)
WV_OFF = 0          # [2, 256] -> 512
BV_OFF = 512        # [256] replicated across partitions
TRI_OFF = 768       # [128] tri[p, q] = 1 if q >= p else 0
CBE_F = 896
# cb_late packed layout (bf16)
WMP_OFF = 0         # [4, 256] -> 1024 (zero-padded mix weights)
SELP_OFF = 1024     # partitions 0-3: [4, 128] selector
G_OFF = 1536        # [4, 4] denominator gather
CBL_F = 1552

_CACHE = {}


def _build_nc():
    f32 = mybir.dt.float32
    bf16 = mybir.dt.bfloat16
    EXP = mybir.ActivationFunctionType.Exp
    ADD = mybir.AluOpType.add

    nc = bacc.Bacc("TRN2", target_bir_lowering=False, debug=False)

    def din(name, shape, dt):
        return nc.dram_tensor(name, shape, dt, kind="ExternalInput").ap()

    hx1 = din("hx1", [128, HX1_F], bf16)
    hx2 = din("hx2", [128, HX2_F], bf16)
    cbe = din("cbe", [128, CBE_F], bf16)
    cbl = din("cbl", [128, CBL_F], bf16)
    y = nc.dram_tensor("y", [2, 128, T], f32, kind="ExternalOutput").ap()

    with tile.TileContext(nc) as tc:
        with (
            tc.tile_pool(name="consts", bufs=1) as consts,
            tc.tile_pool(name="sb", bufs=6) as sb,
            tc.tile_pool(name="ps", bufs=2, space="PSUM") as ps_pool,
            tc.tile_pool(name="lpp", bufs=2, space="PSUM") as lp_pool,
            tc.tile_pool(name="dvp", bufs=2, space="PSUM") as dv_pool,
        ):
            # ---- ACT table load at t~0 (warm act on a memset tile) ----
            wi = consts.tile([128, 1], f32, tag="wi", name="wi")
            nc.vector.memset(wi, 0.0)
            warm = consts.tile([128, 1], f32, tag="warm", name="warm")
            nc.scalar.activation(out=warm, in_=wi, func=EXP)

            # ---- head loads (split so projections start mid-transfer) ----
            hx1s = consts.tile([128, HX1_F], bf16)
            nc.sync.dma_start(out=hx1s, in_=hx1)
            hx2s = consts.tile([128, HX2_F], bf16)
            nc.sync.dma_start(out=hx2s, in_=hx2)
            xbh = {0: hx1s[:, 0:T], 1: hx2s[:, 0:T]}
            cbes = consts.tile([128, CBE_F], bf16)
            nc.gpsimd.dma_start(out=cbes, in_=cbe)
            cbls = consts.tile([128, CBL_F], bf16)  # fetched mid-stream

            wv_v = cbes[:, WV_OFF:WV_OFF + 512].rearrange("p (a j) -> p a j", j=256)
            bv_v = cbes[:, BV_OFF:BV_OFF + 256]
            tri_v = cbes[:, TRI_OFF:TRI_OFF + 128]
            wmp_v = cbls[:, WMP_OFF:WMP_OFF + 1024].rearrange("p (i j) -> p i j", j=256)
            selp_v = cbls[0:4, SELP_OFF:SELP_OFF + 512].rearrange("p (i r) -> p i r", r=128)
            g_v = cbls[:, G_OFF:G_OFF + 16].rearrange("p (i m) -> p i m", m=4)
            wk2v = hx2s[:, 1280:1536].rearrange("p (a j) -> p a j", j=128)
            wq2v = hx2s[:, 1536:1792].rearrange("p (a j) -> p a j", j=128)
            wAPs = {
                "wk": (hx1s[:, 1024:1152], hx2s[:, 1024:1152]),
                "wq": (hx1s[:, 1152:1280], hx2s[:, 1152:1280]),
                "wk2": (wk2v[:, 0, :], wk2v[:, 1, :]),
                "wq2": (wq2v[:, 0, :], wq2v[:, 1, :]),
            }
            bja = consts.tile([128, 6], f32, tag="bja", name="bja")
            nc.vector.tensor_copy(out=bja, in_=hx1s[:, 1280:1286])
            bjt_sb = bja[:, 0:4]
            bm2_sb = bja[:, 4:6]

            # dummy matmuls: warm the PE clock gate while the hx DMA lands
            scr = consts.tile([128, 512], bf16, tag="scr", name="scr")
            nc.vector.memset(scr, 0.125)
            dmy = dv_pool.tile([128, 512], f32, tag="dv", name="dmy")
            for _ in range(12):
                nc.tensor.matmul(out=dmy, lhsT=scr[:, 0:128], rhs=scr,
                                 start=True, stop=True)

            qkT = consts.tile([128, 4, T], bf16)  # 0-1: kT g0/g1, 2-3: qT g0/g1
            vsb = consts.tile([128, 8, H, 33], bf16)  # [p, s_tile, head, v|1]
            nc.vector.memset(vsb[:, :, :, 32:33], 1.0)

            # ---- projections ----
            IDENT = mybir.ActivationFunctionType.Identity

            def proj_group(dst, wname, tch, bias_eng="vector"):
                wk0, wk1 = wAPs[wname]
                ts_ = slice(tch * 512, (tch + 1) * 512)
                p = dv_pool.tile([128, 512], f32, tag="dv", name="pp")
                nc.tensor.matmul(out=p, lhsT=wk0, rhs=xbh[0][:, ts_],
                                 start=True, stop=False)
                nc.tensor.matmul(out=p, lhsT=wk1, rhs=xbh[1][:, ts_],
                                 start=False, stop=True)
                if bias_eng == "scalar":
                    nc.scalar.activation(out=qkT[:, dst, ts_], in_=p,
                                         func=IDENT, bias=bjt_sb[:, dst:dst + 1])
                else:
                    nc.vector.tensor_scalar(
                        out=qkT[:, dst, ts_], in0=p,
                        scalar1=bjt_sb[:, dst:dst + 1], scalar2=None, op0=ADD)

            def v_group(st):
                ss = slice(st * 128, (st + 1) * 128)
                p = dv_pool.tile([128, 512], f32, tag="dv", name="pv")[:, 0:256]
                nc.tensor.matmul(out=p, lhsT=xbh[0][:, ss], rhs=wv_v[:, 0, :],
                                 start=True, stop=False)
                nc.tensor.matmul(out=p, lhsT=xbh[1][:, ss], rhs=wv_v[:, 1, :],
                                 start=False, stop=True)
                nc.vector.tensor_add(
                    out=vsb[:, st, :, 0:32],
                    in0=p.rearrange("p (h e) -> p h e", e=32),
                    in1=bv_v.rearrange("p (h e) -> p h e", e=32))

            # eager: only what chunk-0 group-0 needs (kT bias on the
            # still-idle ScalarE so the two bias ops run in parallel)
            proj_group(0, "wk", 0, bias_eng="scalar")
            proj_group(2, "wq", 0)

            # ---- normalization / mix pipeline pieces ----
            ou = {}      # (c, g) -> [pair] sbuf bf16 copies of po
            attnT = {}   # (c, g, pair) -> sbuf bf16 normalized
            rcb = {}     # (c, g) -> [4, 512] bf16 reciprocal denominators

            def emit_po_copies(c, g, po):
                t = [sb.tile([128, 512], bf16, tag="ou", name="ou", bufs=4)
                     for _ in range(2)]
                for pair in range(2):
                    nc.vector.tensor_copy(out=t[pair], in_=po[pair])
                ou[(c, g)] = t

            def emit_rs(c, g):
                rs = dv_pool.tile([128, 512], f32, tag="dv", name="rs")[0:4, :]
                for pair in range(2):
                    nc.tensor.matmul(out=rs, lhsT=g_v[:, 2 * g + pair, :],
                                     rhs=ou[(c, g)][pair],
                                     start=(pair == 0), stop=(pair == 1))
                rcf = sb.tile([4, 512], f32, tag="rcf", name="rcf", bufs=2)
                nc.vector.reciprocal_approx_fast(out=rcf, in_=rs)
                rc = sb.tile([4, 512], bf16, tag="rcb", name="rcb", bufs=2)
                nc.vector.tensor_copy(out=rc, in_=rcf)
                rcb[(c, g)] = rc

            def emit_norm(c, g, pair):
                bc = dv_pool.tile([128, 512], f32, tag="dv", name="bc")
                nc.tensor.matmul(out=bc, lhsT=selp_v[:, 2 * g + pair, :],
                                 rhs=rcb[(c, g)], start=True, stop=True)
                at = sb.tile([128, 512], bf16, tag="at", name="at", bufs=4)
                nc.vector.tensor_mul(out=at, in0=ou[(c, g)][pair], in1=bc)
                attnT[(c, g, pair)] = at

            def emit_mix(c, c2t):
                cs = slice(c * 512, (c + 1) * 512)
                c2s = slice(c2t * 128, (c2t + 1) * 128)
                if c == 0:
                    mp = dv_pool.tile([128, 512], f32, tag="dv", name="mp")
                else:
                    mp = lp_pool.tile([128, 2, 512], f32, tag="lp", name="mp")[:, 0, :]
                for i, (g, pair) in enumerate(((0, 0), (0, 1), (1, 0), (1, 1))):
                    nc.tensor.matmul(out=mp, lhsT=wmp_v[:, 2 * g + pair, c2s],
                                     rhs=attnT[(c, g, pair)],
                                     start=(i == 0), stop=(i == 3))
                os_ = sb.tile([128, 512], f32, tag="os", name="os")
                nc.vector.scalar_tensor_tensor(
                    out=os_, in0=mp, scalar=bm2_sb[:, c2t:c2t + 1],
                    in1=xbh[c2t][:, cs], op0=ADD, op1=ADD)
                nc.sync.dma_start(out=y.rearrange("a p t -> p a t")[:, c2t, cs], in_=os_)

            def fetch_cbl():
                # data-dep on vsb written at (0,0,2) keeps the scheduler from
                # hoisting this 400KB DMA into the congested head
                nc.vector.tensor_copy(out=cbls[0:1, 0:1], in_=vsb[0:1, 2, 0, 0:1])
                nc.sync.dma_start(out=cbls, in_=cbl)

            def warm_burst(n=3):
                t = dv_pool.tile([128, 512], f32, tag="dv", name="wb")
                for _ in range(n):
                    nc.tensor.matmul(out=t, lhsT=scr[:, 0:128], rhs=scr,
                                     start=True, stop=True)

            def warm_tick():
                t = dv_pool.tile([128, 512], f32, tag="dv", name="wt")
                nc.tensor.matmul(out=t[:, 0:128], lhsT=scr[:, 0:128],
                                 rhs=scr[:, 0:128], start=True, stop=True)

            def pg(*a):
                return lambda: proj_group(*a)

            def vg(st):
                return lambda: v_group(st)

            # per-(c,g,pos) extra work, emitted in the pair-1 PE-idle
            # window; heavier items sit in the full-tile (high-slack) slots
            extras = {
                (0, 0, 0): [vg(0), pg(1, "wk2", 0)],
                (0, 0, 1): [vg(1), pg(3, "wq2", 0)],
                (0, 0, 2): [vg(2), fetch_cbl],
                (0, 0, 3): [vg(3)],
                (0, 1, 0): [warm_burst, pg(2, "wq", 1)],
                (0, 1, 1): [lambda: emit_rs(0, 0)],
                (0, 1, 2): [lambda: emit_norm(0, 0, 0)],
                (0, 1, 3): [lambda: emit_norm(0, 0, 1)],
                (1, 0, 0): [warm_burst, vg(4)],
                (1, 0, 1): [vg(5), pg(0, "wk", 1)],
                (1, 0, 2): [vg(6)],
                (1, 0, 3): [vg(7), pg(3, "wq2", 1)],
                (1, 0, 4): [pg(1, "wk2", 1)],
                (1, 0, 5): [lambda: emit_rs(0, 1)],
                (1, 0, 6): [lambda: emit_norm(0, 1, 0)],
                (1, 0, 7): [lambda: emit_norm(0, 1, 1)],
                (1, 1, 1): [warm_burst, lambda: emit_mix(0, 0)],
                (1, 1, 2): [lambda: emit_mix(0, 1)],
                (1, 1, 3): [lambda: emit_rs(1, 0)],
                (1, 1, 5): [lambda: emit_norm(1, 0, 0)],
                (1, 1, 6): [lambda: emit_norm(1, 0, 1)],
            }

            # ---- attention ----
            pending_av = []
            due = []
            for c in range(2):
                for g in range(2):
                    po = {
                        0: ps_pool.tile([128, 512], f32, tag="po", name="po0"),
                        1: ps_pool.tile([128, 512], f32, tag="po", name="po1"),
                    }
                    nst = 4 * c + 4
                    st_order = list(range(nst))
                    if c == 1 and g == 1:
                        # diag (masked) tiles mid-group; finish on unmasked
                        st_order = [0, 4, 5, 6, 7, 1, 2, 3]
                    for pos, st in enumerate(st_order):
                        tlo = 128 * st - 512 * c
                        diag = tlo >= 0
                        if not diag:
                            tlo = 0
                        for pair in range(2):
                            lp = lp_pool.tile([128, 2, 512], f32, tag="lp", name="lp")
                            E = sb.tile([128, 2, 512], bf16, tag="E", name="E", bufs=8)
                            for h2 in range(2):
                                hl = 2 * pair + h2
                                rp = 32 * hl
                                kT_l = qkT[rp:rp + 32, g, st * 128:(st + 1) * 128]
                                qg = qkT[rp:rp + 32, 2 + g, :]
                                qlo = tlo
                                nc.tensor.matmul(
                                    out=lp[:, h2, qlo:512], lhsT=kT_l,
                                    rhs=qg[:, c * 512 + qlo:(c + 1) * 512],
                                    start=True, stop=True,
                                    tile_position=(rp, 0))
                            if len(pending_av) >= 2:
                                pending_av.pop(0)()
                            if pair == 1:
                                for fn in due:
                                    fn()
                                due = []
                                for fn in extras.get((c, g, pos), ()):
                                    fn()
                                warm_tick()
                            if tlo == 0:
                                nc.scalar.activation(
                                    out=E.rearrange("p a t -> p (a t)"),
                                    in_=lp.rearrange("p a t -> p (a t)"),
                                    func=EXP)
                            else:
                                nc.scalar.activation(
                                    out=E[:, :, tlo:512], in_=lp[:, :, tlo:512],
                                    func=EXP)
                            if diag:
                                tri_b = bass.AP(
                                    tensor=tri_v.tensor, offset=tri_v.offset,
                                    ap=[list(tri_v.ap[0]), [0, 2]]
                                       + list(tri_v.ap[1:]))
                                nc.gpsimd.tensor_mul(
                                    out=E[:, :, tlo:tlo + 128],
                                    in0=E[:, :, tlo:tlo + 128],
                                    in1=tri_b)
                            def av(po_=po, pair_=pair, st_=st, g_=g,
                                   tlo_=tlo, E_=E, pos_=pos, last_=nst - 1):
                                for h2 in range(2):
                                    hl = 2 * pair_ + h2
                                    h = 4 * g_ + hl
                                    nc.tensor.matmul(
                                        out=po_[pair_][64 * h2:64 * h2 + 33, tlo_:512],
                                        lhsT=vsb[:, st_, h, :],
                                        rhs=E_[:, h2, tlo_:512],
                                        start=(pos_ == 0), stop=(pos_ == last_),
                                        skip_group_check=True,
                                        tile_position=(0, 64 * h2))
                            pending_av.append(av)
                    due.append(
                        lambda po_=po, c_=c, g_=g: emit_po_copies(c_, g_, po_))

            # final tail: flush deferred AVs, last po copies, then
            # (c1, g1) normalization + mix
            for fn in pending_av:
                fn()
            for fn in due:
                fn()
            emit_rs(1, 1)
            emit_norm(1, 1, 0)
            emit_norm(1, 1, 1)
            emit_mix(1, 0)
            emit_mix(1, 1)

    nc.compile()
    return nc


def _host_inputs(image, w_kqv, b_kqv, w_mix, b_mix):
    s = np.float32(1.0 / np.sqrt(D))
    wk_full = w_kqv[:, :256].astype(np.float32)
    wq_full = (w_kqv[:, 256:512] * s).astype(np.float32)
    wv_full = w_kqv[:, 512:].astype(np.float32)
    bk = b_kqv[:256].astype(np.float32)
    bq = (b_kqv[256:512] * s).astype(np.float32)
    bv = b_kqv[512:].astype(np.float32)

    def khalves(w, jlo):  # [p, khalf, j] flattened to [p, 256]
        return w[:, jlo:jlo + 128].reshape(2, 128, 128).transpose(1, 0, 2).reshape(128, 256)

    wk_h = khalves(wk_full, 0)      # [p, 2*128]: kh0 | kh1
    wq_h = khalves(wq_full, 0)
    h1w = np.zeros((128, HX1_F - T), np.float32)
    h1w[:, 0:128] = wk_h[:, 0:128]
    h1w[:, 128:256] = wq_h[:, 0:128]
    h1w[:, 256:260] = np.stack([bk[0:128], bk[128:256], bq[0:128], bq[128:256]], axis=1)
    h1w[:, 260:262] = np.asarray(b_mix, np.float32).reshape(2, 128).T
    h2w = np.zeros((128, HX2_F - T), np.float32)
    h2w[:, 0:128] = wk_h[:, 128:256]
    h2w[:, 128:256] = wq_h[:, 128:256]
    h2w[:, 256:512] = khalves(wk_full, 128)
    h2w[:, 512:768] = khalves(wq_full, 128)

    cbe = np.zeros((128, CBE_F), np.float32)
    cbe[:, WV_OFF:WV_OFF + 512] = wv_full.reshape(2, 128, 256).transpose(1, 0, 2).reshape(128, 512)
    cbe[:, BV_OFF:BV_OFF + 256] = bv[None, :]
    cbe[:, TRI_OFF:TRI_OFF + 128] = (np.arange(128)[None, :] >= np.arange(128)[:, None])

    cbl = np.zeros((128, CBL_F), np.float32)
    wmp = np.zeros((128, 4, 256), np.float32)
    selp = np.zeros((128, 4, 128), np.float32)
    gm = np.zeros((128, 4, 4), np.float32)
    for g in range(2):
        for pair in range(2):
            idx = 2 * g + pair
            for h2 in range(2):
                h = 4 * g + 2 * pair + h2
                wmp[64 * h2:64 * h2 + 32, idx, :] = w_mix[32 * h:32 * h + 32, :]
            selp[2 * pair, idx, 0:32] = 1.0
            selp[2 * pair + 1, idx, 64:96] = 1.0
            gm[32, idx, 2 * pair] = 1.0
            gm[96, idx, 2 * pair + 1] = 1.0
    cbl[:, WMP_OFF:WMP_OFF + 1024] = wmp.reshape(128, 1024)
    cbl[:, SELP_OFF:SELP_OFF + 512] = selp.reshape(128, 512)
    cbl[:, G_OFF:G_OFF + 16] = gm.reshape(128, 16)

    common = {
        "cbe": cbe.astype(BF),
        "cbl": cbl.astype(BF),
    }
    h1b = h1w.astype(BF)
    h2b = h2w.astype(BF)
    in_maps = []
    for i in range(N_CORES):
        x = image[i].reshape(2, 128, T).transpose(1, 0, 2)  # [p, half, t]
        xb_ = x.astype(BF)
        in_maps.append({
            **common,
            "hx1": np.ascontiguousarray(np.concatenate([xb_[:, 0, :], h1b], axis=1)),
            "hx2": np.ascontiguousarray(np.concatenate([xb_[:, 1, :], h2b], axis=1)),
        })
    return in_maps


def _run(inputs, trace=False):
    if "nc" not in _CACHE:
        _CACHE["nc"] = _build_nc()
    nc = _CACHE["nc"]
    in_maps = _host_inputs(
        np.asarray(inputs["image"], np.float32),
        np.asarray(inputs["w_kqv"], np.float32),
        np.asarray(inputs["b_kqv"], np.float32),
        np.asarray(inputs["w_mix"], np.float32),
        np.asarray(inputs["b_mix"], np.float32),
    )
    res = run_bass_kernel_spmd(nc, in_maps, list(range(N_CORES)), trace=trace)
    out = np.stack(
        [np.asarray(res.results[i]["y"]).reshape(C, 32, 32) for i in range(N_CORES)]
    ).astype(np.float32)
    return out, res


def kernel(**inputs):
    out, _ = _run(inputs, trace=False)
    return out


# revision 19
# speedup vs baseline: 1.0817x; 1.0817x over previous
"""PixelAttention Trainium2 kernel.

Computes, for each batch image (data-parallel, one image per NeuronCore):
    seq  = image.reshape(C, T).T            # [T, C], T = 32*32
    kqv  = seq @ w_kqv + b_kqv
    per-head causal attention (8 heads, head_dim 32), softmax over keys
    out  = mix(attn) + b_mix + image

The ScalarE exp stream (~4.7M causal logits at 1 elem/cycle/lane) is the
roofline; everything else is organized to hide under it:
  - ScalarE runs nothing but the 48 exp calls (+1 warm-up that triggers the
    ACT table load at t~0 from a memset input).
  - QK^T logits land transposed L[s, t]; 4 heads row-packed (tile_position)
    per s-tile; the lp PSUM pool (2 slots x 2 banks) is reserved for the
    QK->exp stream alone.
  - AV matmuls ride a 2-deep FIFO, emitted after the next-but-one pair's
    QK, so exp->mask->AV chains never sit between an exp and the following
    QK on the in-order PE.
  - All non-QK PSUM transients (V projection, woven kq projections,
    denominator gather, reciprocal broadcast, mix) use a separate 1-bank
    "dv" pool; the work is spread across tiles via an `extras` schedule
    that lands in PE-idle windows, balanced against each phase's exp time.
  - Causal diag-block masking = GpSimd multiply by a 0/1 triangle.
  - AV accumulates [V | 1] (ones row = softmax denominator) col-packed
    2 heads/matmul; normalization (cast, gather-matmul, reciprocal,
    selector broadcast, multiply) + padded mix are staggered so only the
    last group's tail is exposed.
  - Head loads: one packed DMA (x + all projection weights, 6KB/partition
    lines); dummy matmuls warm the PE clock gate during the transfer; late
    constants are fetched mid-stream behind a data dependency.
"""

import numpy as np
import ml_dtypes

import concourse.bass as bass
import concourse.tile as tile
from concourse import bacc, mybir
from concourse.bass_utils import run_bass_kernel_spmd

BF = ml_dtypes.bfloat16
T, C, H, D = 1024, 256, 8, 32
N_CORES = 8

# hx1 (bf16): x half0 (1024) | wk kh0 (128) | wq kh0 (128) | bjt 4 | bm2 2
HX1_F = 1286
# hx2 (bf16): x half1 (1024) | wk kh1 (128) | wq kh1 (128) | wk2 [2,128] | wq2 [2,128]
HX2_F = 1792
# cb_early packed layout (bf16)
WV_OFF = 0          # [2, 256] -> 512
BV_OFF = 512        # [256] replicated across partitions
TRI_OFF = 768       # [128] tri[p, q] = 1 if q >= p else 0
CBE_F = 896
# cb_late packed layout (bf16)
WMP_OFF = 0         # [4, 256] -> 1024 (zero-padded mix weights)
SELP_OFF = 1024     # partitions 0-3: [4, 128] selector
G_OFF = 1536        # [4, 4] denominator gather
CBL_F = 1552

_CACHE = {}


def _build_nc():
    f32 = mybir.dt.float32
    bf16 = mybir.dt.bfloat16
    EXP = mybir.ActivationFunctionType.Exp
    ADD = mybir.AluOpType.add

    nc = bacc.Bacc("TRN2", target_bir_lowering=False, debug=False)

    def din(name, shape, dt):
        return nc.dram_tensor(name, shape, dt, kind="ExternalInput").ap()

    hx1 = din("hx1", [128, HX1_F], bf16)
    hx2 = din("hx2", [128, HX2_F], bf16)
    cbe = din("cbe", [128, CBE_F], bf16)
    cbl = din("cbl", [128, CBL_F], bf16)
    y = nc.dram_tensor("y", [2, 128, T], f32, kind="ExternalOutput").ap()

    with tile.TileContext(nc) as tc:
        with (
            tc.tile_pool(name="consts", bufs=1) as consts,
            tc.tile_pool(name="sb", bufs=6) as sb,
            tc.tile_pool(name="ps", bufs=2, space="PSUM") as ps_pool,
            tc.tile_pool(name="lpp", bufs=2, space="PSUM") as lp_pool,
            tc.tile_pool(name="dvp", bufs=2, space="PSUM") as dv_pool,
        ):
            # ---- ACT table load at t~0 (warm act on a memset tile) ----
            wi = consts.tile([128, 1], f32, tag="wi", name="wi")
            nc.vector.memset(wi, 0.0)
            warm = consts.tile([128, 1], f32, tag="warm", name="warm")
            nc.scalar.activation(out=warm, in_=wi, func=EXP)

            # ---- head loads (split so projections start mid-transfer) ----
            hx1s = consts.tile([128, HX1_F], bf16)
            nc.sync.dma_start(out=hx1s, in_=hx1)
            hx2s = consts.tile([128, HX2_F], bf16)
            nc.sync.dma_start(out=hx2s, in_=hx2)
            xbh = {0: hx1s[:, 0:T], 1: hx2s[:, 0:T]}
            cbes = consts.tile([128, CBE_F], bf16)
            nc.gpsimd.dma_start(out=cbes, in_=cbe)
            cbls = consts.tile([128, CBL_F], bf16)  # fetched mid-stream

            wv_v = cbes[:, WV_OFF:WV_OFF + 512].rearrange("p (a j) -> p a j", j=256)
            bv_v = cbes[:, BV_OFF:BV_OFF + 256]
            tri_v = cbes[:, TRI_OFF:TRI_OFF + 128]
            wmp_v = cbls[:, WMP_OFF:WMP_OFF + 1024].rearrange("p (i j) -> p i j", j=256)
            selp_v = cbls[0:4, SELP_OFF:SELP_OFF + 512].rearrange("p (i r) -> p i r", r=128)
            g_v = cbls[:, G_OFF:G_OFF + 16].rearrange("p (i m) -> p i m", m=4)
            wk2v = hx2s[:, 1280:1536].rearrange("p (a j) -> p a j", j=128)
            wq2v = hx2s[:, 1536:1792].rearrange("p (a j) -> p a j", j=128)
            wAPs = {
                "wk": (hx1s[:, 1024:1152], hx2s[:, 1024:1152]),
                "wq": (hx1s[:, 1152:1280], hx2s[:, 1152:1280]),
                "wk2": (wk2v[:, 0, :], wk2v[:, 1, :]),
                "wq2": (wq2v[:, 0, :], wq2v[:, 1, :]),
            }
            bja = consts.tile([128, 6], f32, tag="bja", name="bja")
            nc.vector.tensor_copy(out=bja, in_=hx1s[:, 1280:1286])
            bjt_sb = bja[:, 0:4]
            bm2_sb = bja[:, 4:6]

            # dummy matmuls: warm the PE clock gate while the hx DMA lands
            scr = consts.tile([128, 512], bf16, tag="scr", name="scr")
            nc.vector.memset(scr, 0.125)
            dmy = dv_pool.tile([128, 512], f32, tag="dv", name="dmy")
            for _ in range(12):
                nc.tensor.matmul(out=dmy, lhsT=scr[:, 0:128], rhs=scr,
                                 start=True, stop=True)

            qkT = consts.tile([128, 4, T], bf16)  # 0-1: kT g0/g1, 2-3: qT g0/g1
            vsb = consts.tile([128, 8, H, 33], bf16)  # [p, s_tile, head, v|1]
            nc.vector.memset(vsb[:, :, :, 32:33], 1.0)

            # ---- projections ----
            IDENT = mybir.ActivationFunctionType.Identity

            def proj_group(dst, wname, tch, bias_eng="vector"):
                wk0, wk1 = wAPs[wname]
                ts_ = slice(tch * 512, (tch + 1) * 512)
                p = dv_pool.tile([128, 512], f32, tag="dv", name="pp")
                nc.tensor.matmul(out=p, lhsT=wk0, rhs=xbh[0][:, ts_],
                                 start=True, stop=False)
                nc.tensor.matmul(out=p, lhsT=wk1, rhs=xbh[1][:, ts_],
                                 start=False, stop=True)
                if bias_eng == "scalar":
                    nc.scalar.activation(out=qkT[:, dst, ts_], in_=p,
                                         func=IDENT, bias=bjt_sb[:, dst:dst + 1])
                else:
                    nc.vector.tensor_scalar(
                        out=qkT[:, dst, ts_], in0=p,
                        scalar1=bjt_sb[:, dst:dst + 1], scalar2=None, op0=ADD)

            def v_group(st):
                ss = slice(st * 128, (st + 1) * 128)
                p = dv_pool.tile([128, 512], f32, tag="dv", name="pv")[:, 0:256]
                nc.tensor.matmul(out=p, lhsT=xbh[0][:, ss], rhs=wv_v[:, 0, :],
                                 start=True, stop=False)
                nc.tensor.matmul(out=p, lhsT=xbh[1][:, ss], rhs=wv_v[:, 1, :],
                                 start=False, stop=True)
                nc.vector.tensor_add(
                    out=vsb[:, st, :, 0:32],
                    in0=p.rearrange("p (h e) -> p h e", e=32),
                    in1=bv_v.rearrange("p (h e) -> p h e", e=32))

            # eager: only what chunk-0 group-0 needs (kT bias on the
            # still-idle ScalarE so the two bias ops run in parallel)
            proj_group(0, "wk", 0, bias_eng="scalar")
            proj_group(2, "wq", 0)

            # ---- normalization / mix pipeline pieces ----
            ou = {}      # (c, g) -> [pair] sbuf bf16 copies of po
            attnT = {}   # (c, g, pair) -> sbuf bf16 normalized
            rcb = {}     # (c, g) -> [4, 512] bf16 reciprocal denominators

            def emit_po_copies(c, g, po):
                t = [sb.tile([128, 512], bf16, tag="ou", name="ou", bufs=4)
                     for _ in range(2)]
                for pair in range(2):
                    nc.vector.tensor_copy(out=t[pair], in_=po[pair])
                ou[(c, g)] = t

            def emit_rs(c, g):
                rs = dv_pool.tile([128, 512], f32, tag="dv", name="rs")[0:4, :]
                for pair in range(2):
                    nc.tensor.matmul(out=rs, lhsT=g_v[:, 2 * g + pair, :],
                                     rhs=ou[(c, g)][pair],
                                     start=(pair == 0), stop=(pair == 1))
                rcf = sb.tile([4, 512], f32, tag="rcf", name="rcf", bufs=2)
                nc.vector.reciprocal_approx_fast(out=rcf, in_=rs)
                rc = sb.tile([4, 512], bf16, tag="rcb", name="rcb", bufs=2)
                nc.vector.tensor_copy(out=rc, in_=rcf)
                rcb[(c, g)] = rc

            def emit_norm(c, g, pair):
                bc = dv_pool.tile([128, 512], f32, tag="dv", name="bc")
                nc.tensor.matmul(out=bc, lhsT=selp_v[:, 2 * g + pair, :],
                                 rhs=rcb[(c, g)], start=True, stop=True)
                at = sb.tile([128, 512], bf16, tag="at", name="at", bufs=4)
                nc.vector.tensor_mul(out=at, in0=ou[(c, g)][pair], in1=bc)
                attnT[(c, g, pair)] = at

            def emit_mix(c, c2t):
                cs = slice(c * 512, (c + 1) * 512)
                c2s = slice(c2t * 128, (c2t + 1) * 128)
                if c == 0:
                    mp = dv_pool.tile([128, 512], f32, tag="dv", name="mp")
                else:
                    mp = lp_pool.tile([128, 2, 512], f32, tag="lp", name="mp")[:, 0, :]
                for i, (g, pair) in enumerate(((0, 0), (0, 1), (1, 0), (1, 1))):
                    nc.tensor.matmul(out=mp, lhsT=wmp_v[:, 2 * g + pair, c2s],
                                     rhs=attnT[(c, g, pair)],
                                     start=(i == 0), stop=(i == 3))
                os_ = sb.tile([128, 512], f32, tag="os", name="os")
                nc.vector.scalar_tensor_tensor(
                    out=os_, in0=mp, scalar=bm2_sb[:, c2t:c2t + 1],
                    in1=xbh[c2t][:, cs], op0=ADD, op1=ADD)
                nc.sync.dma_start(out=y.rearrange("a p t -> p a t")[:, c2t, cs], in_=os_)

            def fetch_cbl():
                # data-dep on vsb written at (0,0,2) keeps the scheduler from
                # hoisting this 400KB DMA into the congested head
                nc.vector.tensor_copy(out=cbls[0:1, 0:1], in_=vsb[0:1, 2, 0, 0:1])
                nc.sync.dma_start(out=cbls, in_=cbl)

            def warm_burst(n=3):
                t = dv_pool.tile([128, 512], f32, tag="dv", name="wb")
                for _ in range(n):
                    nc.tensor.matmul(out=t, lhsT=scr[:, 0:128], rhs=scr,
                                     start=True, stop=True)

            def warm_tick():
                warm_burst(1)

            def pg(*a):
                return lambda: proj_group(*a)

            def vg(st):
                return lambda: v_group(st)

            # per-(c,g,pos) extra work, emitted in the pair-1 PE-idle
            # window; heavier items sit in the full-tile (high-slack) slots
            extras = {
                (0, 0, 0): [vg(0), pg(1, "wk2", 0)],
                (0, 0, 1): [vg(1), pg(3, "wq2", 0)],
                (0, 0, 2): [vg(2), fetch_cbl],
                (0, 0, 3): [vg(3)],
                (0, 1, 0): [warm_burst, pg(2, "wq", 1)],
                (0, 1, 1): [lambda: emit_rs(0, 0)],
                (0, 1, 2): [lambda: emit_norm(0, 0, 0)],
                (0, 1, 3): [lambda: emit_norm(0, 0, 1)],
                (1, 0, 0): [warm_burst, vg(4)],
                (1, 0, 1): [vg(5), pg(0, "wk", 1)],
                (1, 0, 2): [vg(6)],
                (1, 0, 3): [vg(7), pg(3, "wq2", 1)],
                (1, 0, 4): [pg(1, "wk2", 1)],
                (1, 0, 5): [lambda: emit_rs(0, 1)],
                (1, 0, 6): [lambda: emit_norm(0, 1, 0)],
                (1, 0, 7): [lambda: emit_norm(0, 1, 1)],
                (1, 1, 1): [warm_burst, lambda: emit_mix(0, 0)],
                (1, 1, 2): [lambda: emit_mix(0, 1)],
                (1, 1, 3): [lambda: emit_rs(1, 0)],
                (1, 1, 5): [lambda: emit_norm(1, 0, 0)],
                (1, 1, 6): [lambda: emit_norm(1, 0, 1)],
            }

            # ---- attention ----
            pending_av = []
            due = []
            for c in range(2):
                for g in range(2):
                    po = {
                        0: ps_pool.tile([128, 512], f32, tag="po", name="po0"),
                        1: ps_pool.tile([128, 512], f32, tag="po", name="po1"),
                    }
                    nst = 4 * c + 4
                    st_order = list(range(nst))
                    if c == 1 and g == 1:
                        # diag (masked) tiles mid-group; finish on unmasked
                        st_order = [0, 4, 5, 6, 7, 1, 2, 3]
                    for pos, st in enumerate(st_order):
                        tlo = 128 * st - 512 * c
                        diag = tlo >= 0
                        if not diag:
                            tlo = 0
                        for pair in range(2):
                            lp = lp_pool.tile([128, 2, 512], f32, tag="lp", name="lp")
                            E = sb.tile([128, 2, 512], bf16, tag="E", name="E", bufs=8)
                            for h2 in range(2):
                                hl = 2 * pair + h2
                                rp = 32 * hl
                                kT_l = qkT[rp:rp + 32, g, st * 128:(st + 1) * 128]
                                qg = qkT[rp:rp + 32, 2 + g, :]
                                qlo = 0
                                nc.tensor.matmul(
                                    out=lp[:, h2, qlo:512], lhsT=kT_l,
                                    rhs=qg[:, c * 512 + qlo:(c + 1) * 512],
                                    start=True, stop=True,
                                    tile_position=(rp, 0))
                            if len(pending_av) >= 2:
                                pending_av.pop(0)()
                            if pair == 1:
                                for fn in due:
                                    fn()
                                due = []
                                for fn in extras.get((c, g, pos), ()):
                                    fn()
                                warm_tick()
                            if tlo == 0:
                                nc.scalar.activation(
                                    out=E.rearrange("p a t -> p (a t)"),
                                    in_=lp.rearrange("p a t -> p (a t)"),
                                    func=EXP)
                            else:
                                nc.scalar.activation(
                                    out=E[:, :, tlo:512], in_=lp[:, :, tlo:512],
                                    func=EXP)
                            if diag:
                                tri_b = bass.AP(
                                    tensor=tri_v.tensor, offset=tri_v.offset,
                                    ap=[list(tri_v.ap[0]), [0, 2]]
                                       + list(tri_v.ap[1:]))
                                nc.gpsimd.tensor_mul(
                                    out=E[:, :, tlo:tlo + 128],
                                    in0=E[:, :, tlo:tlo + 128],
                                    in1=tri_b)
                            def av(po_=po, pair_=pair, st_=st, g_=g,
                                   tlo_=tlo, E_=E, pos_=pos, last_=nst - 1):
                                for h2 in range(2):
                                    hl = 2 * pair_ + h2
                                    h = 4 * g_ + hl
                                    nc.tensor.matmul(
                                        out=po_[pair_][64 * h2:64 * h2 + 33, tlo_:512],
                                        lhsT=vsb[:, st_, h, :],
                                        rhs=E_[:, h2, tlo_:512],
                                        start=(pos_ == 0), stop=(pos_ == last_),
                                        skip_group_check=True,
                                        tile_position=(0, 64 * h2))
                            pending_av.append(av)
                    due.append(
                        lambda po_=po, c_=c, g_=g: emit_po_copies(c_, g_, po_))

            # final tail: flush deferred AVs, last po copies, then
            # (c1, g1) normalization + mix
            for fn in pending_av:
                fn()
            for fn in due:
                fn()
            emit_rs(1, 1)
            emit_norm(1, 1, 0)
            emit_norm(1, 1, 1)
            emit_mix(1, 0)
            emit_mix(1, 1)

    nc.compile()
    return nc


def _host_inputs(image, w_kqv, b_kqv, w_mix, b_mix):
    s = np.float32(1.0 / np.sqrt(D))
    wk_full = w_kqv[:, :256].astype(np.float32)
    wq_full = (w_kqv[:, 256:512] * s).astype(np.float32)
    wv_full = w_kqv[:, 512:].astype(np.float32)
    bk = b_kqv[:256].astype(np.float32)
    bq = (b_kqv[256:512] * s).astype(np.float32)
    bv = b_kqv[512:].astype(np.float32)

    def khalves(w, jlo):  # [p, khalf, j] flattened to [p, 256]
        return w[:, jlo:jlo + 128].reshape(2, 128, 128).transpose(1, 0, 2).reshape(128, 256)

    wk_h = khalves(wk_full, 0)      # [p, 2*128]: kh0 | kh1
    wq_h = khalves(wq_full, 0)
    h1w = np.zeros((128, HX1_F - T), np.float32)
    h1w[:, 0:128] = wk_h[:, 0:128]
    h1w[:, 128:256] = wq_h[:, 0:128]
    h1w[:, 256:260] = np.stack([bk[0:128], bk[128:256], bq[0:128], bq[128:256]], axis=1)
    h1w[:, 260:262] = np.asarray(b_mix, np.float32).reshape(2, 128).T
    h2w = np.zeros((128, HX2_F - T), np.float32)
    h2w[:, 0:128] = wk_h[:, 128:256]
    h2w[:, 128:256] = wq_h[:, 128:256]
    h2w[:, 256:512] = khalves(wk_full, 128)
    h2w[:, 512:768] = khalves(wq_full, 128)

    cbe = np.zeros((128, CBE_F), np.float32)
    cbe[:, WV_OFF:WV_OFF + 512] = wv_full.reshape(2, 128, 256).transpose(1, 0, 2).reshape(128, 512)
    cbe[:, BV_OFF:BV_OFF + 256] = bv[None, :]
    cbe[:, TRI_OFF:TRI_OFF + 128] = (np.arange(128)[None, :] >= np.arange(128)[:, None])

    cbl = np.zeros((128, CBL_F), np.float32)
    wmp = np.zeros((128, 4, 256), np.float32)
    selp = np.zeros((128, 4, 128), np.float32)
    gm = np.zeros((128, 4, 4), np.float32)
    for g in range(2):
        for pair in range(2):
            idx = 2 * g + pair
            for h2 in range(2):
                h = 4 * g + 2 * pair + h2
                wmp[64 * h2:64 * h2 + 32, idx, :] = w_mix[32 * h:32 * h + 32, :]
            selp[2 * pair, idx, 0:32] = 1.0
            selp[2 * pair + 1, idx, 64:96] = 1.0
            gm[32, idx, 2 * pair] = 1.0
            gm[96, idx, 2 * pair + 1] = 1.0
    cbl[:, WMP_OFF:WMP_OFF + 1024] = wmp.reshape(128, 1024)
    cbl[:, SELP_OFF:SELP_OFF + 512] = selp.reshape(128, 512)
    cbl[:, G_OFF:G_OFF + 16] = gm.reshape(128, 16)

    common = {
        "cbe": cbe.astype(BF),
        "cbl": cbl.astype(BF),
    }
    h1b = h1w.astype(BF)
    h2b = h2w.astype(BF)
    in_maps = []
    for i in range(N_CORES):
        x = image[i].reshape(2, 128, T).transpose(1, 0, 2)  # [p, half, t]
        xb_ = x.astype(BF)
        in_maps.append({
            **common,
            "hx1": np.ascontiguousarray(np.concatenate([xb_[:, 0, :], h1b], axis=1)),
            "hx2": np.ascontiguousarray(np.concatenate([xb_[:, 1, :], h2b], axis=1)),
        })
    return in_maps


def _run(inputs, trace=False):
    if "nc" not in _CACHE:
        _CACHE["nc"] = _build_nc()
    nc = _CACHE["nc"]
    in_maps = _host_inputs(
        np.asarray(inputs["image"], np.float32),
        np.asarray(inputs["w_kqv"], np.float32),
        np.asarray(inputs["b_kqv"], np.float32),
        np.asarray(inputs["w_mix"], np.float32),
        np.asarray(inputs["b_mix"], np.float32),
    )
    res = run_bass_kernel_spmd(nc, in_maps, list(range(N_CORES)), trace=trace)
    out = np.stack(
        [np.asarray(res.results[i]["y"]).reshape(C, 32, 32) for i in range(N_CORES)]
    ).astype(np.float32)
    return out, res


def kernel(**inputs):
    out, _ = _run(inputs, trace=False)
    return out
